# revision 1
# baseline (speedup 1.0000x reference)
"""Trainium2 Bass kernel for nn_BlockV2 (conv -> LN -> minGRU -> MLP x4).

Strategy: data-parallel over batch (B=8 -> 8 cores). Per core, activations
are kept in [D_partitions, T_free] layout and streamed through each layer in
chunks of 512 tokens; inter-layer activations ping-pong through DRAM in
fp32 (the late-layer stream is a ~5e-3 variation on an O(1) baseline;
storing it bf16 measures 28% output error - LN divides by the tiny
per-token sigma and amplifies absolute quantization noise ~200x).
Values that carry *relative* information are bf16: the centered LN values,
the conv taps/accumulator, the MLP output m, and the raw input x.
The minGRU recurrence h_t = c_t*h_{t-1} + v_t runs on the VectorE
tensor_tensor_scan instruction (fp32 state), chained across chunks.

Perf structure (v4):
- conv pointwise folded into the GRU input projection on the host
  (FW2 = f_w @ pw, bias into the gate activations): the kh matmul streams
  the depthwise-conv output y directly.
- LN mean sums run as bf16 matmuls over a bf16 copy of x fetched by a
  GpSimd casting DMA (mean over 512 washes out the quantization noise);
  fp32 4-cyc/row stat matmuls remain only for L0's SBUF-resident cv.
- Ln/Exp are routed to the combined natural_log_exp_and_others table set
  (the default per-function choice loads natural_log and exp_and_others
  separately - 3 table swaps per chunk instead of 2).
- Mid-layer-0 chunks interleave into the L0 chunk stream (stagger 4): L0
  alone is PE-light/vector-heavy and left TensorE at 22-67% occupancy for
  350us. fw/pw weight slots alternate by layer parity so the prefetch
  cannot WAR-deadlock against the previous layer's last reads.
- All small parameters are pre-swizzled on the host so DMAs are
  partition-contiguous.
"""
import sys

sys.path.insert(0, "/opt/trn_rl_repo")

from contextlib import ExitStack

import numpy as np
import ml_dtypes

import concourse.bass as bass
import concourse.tile as tile
from concourse import bacc, mybir

# Route Ln/Exp activations to the combined natural_log_exp_and_others table
# set (set order, and thus act_func_set_id, is preserved; only membership of
# the single-anchor sets is trimmed).
import functools
import concourse.hw_specs as _hw_specs

_orig_gat = _hw_specs.get_activation_tables


@functools.cache
def _patched_gat(arch):
    t = {k: set(v) for k, v in _orig_gat(arch).items()}
    comb = "natural_log_exp_and_others"
    if comb in t:
        for name, fns in t.items():
            if name != comb:
                fns.discard(mybir.ActivationFunctionType.Ln)
                fns.discard(mybir.ActivationFunctionType.Exp)
    return t


_hw_specs.get_activation_tables = _patched_gat
bacc.get_activation_tables = _patched_gat

f32 = mybir.dt.float32
bf16 = mybir.dt.bfloat16
Alu = mybir.AluOpType
Act = mybir.ActivationFunctionType
BF = ml_dtypes.bfloat16

B, D, L, K, H = 8, 512, 4, 4, 2048
N_CORES = 8
LN_EPS = 1e-5
P = 128


def build_nc(T=4096, CH=512, has_lnb=False):
    NCH = T // CH
    DT = D // P      # 4 d-tiles
    HT = H // P      # 16 h-tiles
    E2 = 2 * D
    MT2 = E2 // P    # 8 m-tiles of the kh matmul

    nc = bacc.Bacc("TRN2", target_bir_lowering=False, debug=False)

    xT = nc.dram_tensor("xT", [D, T + 3], bf16, kind="ExternalInput")
    fwT = nc.dram_tensor("fwT", [L, P, DT, E2], bf16, kind="ExternalInput")
    pwT = nc.dram_tensor("pwT", [L, P, DT, D], bf16, kind="ExternalInput")
    w1T = nc.dram_tensor("w1T", [L, P, DT, H], bf16, kind="ExternalInput")
    w2T = nc.dram_tensor("w2T", [L, P, HT, D], bf16, kind="ExternalInput")
    dwK = nc.dram_tensor("dwK", [P, L * DT, K], f32, kind="ExternalInput")
    dwb = nc.dram_tensor("dwb", [P, L * DT], f32, kind="ExternalInput")
    pwb = nc.dram_tensor("pwb", [P, L * DT], f32, kind="ExternalInput")
    b1v = nc.dram_tensor("b1v", [P, L * HT], f32, kind="ExternalInput")
    b2v = nc.dram_tensor("b2v", [P, L * DT], f32, kind="ExternalInput")
    lng = nc.dram_tensor("lng", [P, (L + 1) * DT], f32, kind="ExternalInput")
    lnb = nc.dram_tensor("lnb", [P, (L + 1) * DT], f32, kind="ExternalInput")
    kbz = nc.dram_tensor("kbz", [P, L * MT2], f32, kind="ExternalInput")
    kbn = nc.dram_tensor("kbn", [P, L * DT], f32, kind="ExternalInput")
    kbh = nc.dram_tensor("kbh", [P, L * DT], f32, kind="ExternalInput")
    out_t = nc.dram_tensor("out", [D, T], f32, kind="ExternalOutput")
    xs = [nc.dram_tensor(f"xs{i}", [D, T], f32) for i in range(2)]
    xs_bf = nc.dram_tensor("xs_bf", [D, T], bf16)

    def dram3(tensor, c, width):
        return tensor.ap().rearrange("(dt p) t -> p dt t", p=P)[:, :, c * CH: c * CH + width]

    with tile.TileContext(nc) as tc, ExitStack() as ctx:
        sing = ctx.enter_context(tc.tile_pool(name="sing", bufs=1))
        wpool = ctx.enter_context(tc.tile_pool(name="w", bufs=1))
        big = ctx.enter_context(tc.tile_pool(name="big", bufs=7))
        small = ctx.enter_context(tc.tile_pool(name="small", bufs=10))
        hidp = ctx.enter_context(tc.tile_pool(name="hid", bufs=2))
        statp = ctx.enter_context(tc.tile_pool(name="stat", bufs=6))
        keep = ctx.enter_context(tc.tile_pool(name="keep", bufs=4))
        psmm = ctx.enter_context(tc.tile_pool(name="psmm", bufs=4, space="PSUM"))
        psst = ctx.enter_context(tc.tile_pool(name="psst", bufs=2, space="PSUM"))
        psbc = ctx.enter_context(tc.tile_pool(name="psbc", bufs=2, space="PSUM"))

        ones_col = sing.tile([P, 1], bf16)
        nc.vector.memset(ones_col, 1.0)
        ones_colf = sing.tile([P, 1], f32)
        nc.vector.memset(ones_colf, 1.0)
        ones_row = sing.tile([1, P], f32)
        nc.vector.memset(ones_row, 1.0)
        ones_row_bf = sing.tile([1, P], bf16)
        nc.vector.memset(ones_row_bf, 1.0)
        eps1 = sing.tile([1, 1], f32)
        nc.vector.memset(eps1, LN_EPS)
        dw_sb = sing.tile([P, L * DT, K], f32)
        nc.sync.dma_start(out=dw_sb, in_=dwK.ap())
        dwb_sb = sing.tile([P, L * DT], f32)
        nc.sync.dma_start(out=dwb_sb, in_=dwb.ap())
        pwb_sb = sing.tile([P, L * DT], f32)
        nc.sync.dma_start(out=pwb_sb, in_=pwb.ap())
        b1_sb = sing.tile([P, L * HT], f32)
        nc.sync.dma_start(out=b1_sb, in_=b1v.ap())
        b2_sb = sing.tile([P, L * DT], f32)
        nc.sync.dma_start(out=b2_sb, in_=b2v.ap())
        lng_sb = sing.tile([P, (L + 1) * DT], f32)
        nc.sync.dma_start(out=lng_sb, in_=lng.ap())
        lnb_sb = sing.tile([P, (L + 1) * DT], f32)
        nc.sync.dma_start(out=lnb_sb, in_=lnb.ap())
        kbz_sb = sing.tile([P, L * MT2], f32)
        nc.sync.dma_start(out=kbz_sb, in_=kbz.ap())
        kbn_sb = sing.tile([P, L * DT], f32)
        nc.sync.dma_start(out=kbn_sb, in_=kbn.ap())
        kbh_sb = sing.tile([P, L * DT], f32)
        nc.sync.dma_start(out=kbh_sb, in_=kbh.ap())

        def load_w(tag, dram, l, shape):
            t = wpool.tile(shape, bf16, tag=tag, name=f"{tag}_{l}")
            nc.sync.dma_start(out=t, in_=dram.ap()[l])
            return t

        def ln_stats(xq, shift):
            """s0 stats: S- and Q-matmuls on the bf16 tile xq (shifted by
            -0.5 for the fragile mid streams), plus row math for the
            uncentered variance  var = Q/D - (S/D)^2  (benign after the
            shift). Returns (S_sb broadcast row incl. the shift-back, varrow)."""
            xsq = small.tile([P, DT, CH], bf16, tag="small", name="xsq")
            for d in range(DT):
                nc.vector.tensor_mul(xsq[:, d, :], xq[:, d, :], xq[:, d, :])
            S_ps = psst.tile([1, CH], f32, tag="ps_stat", name="S_ps")
            for kt in range(DT):
                nc.tensor.matmul(S_ps[:, :], ones_col, xq[:, kt, :],
                                 start=(kt == 0), stop=(kt == DT - 1))
            Q_ps = psst.tile([1, CH], f32, tag="ps_stat", name="Q_ps")
            for kt in range(DT):
                nc.tensor.matmul(Q_ps[:, :], ones_col, xsq[:, kt, :],
                                 start=(kt == 0), stop=(kt == DT - 1))
            S_raw = statp.tile([1, CH], f32, tag="stat", name="S_raw")
            nc.vector.tensor_copy(out=S_raw[:, :], in_=S_ps[:, :])
            varrow = statp.tile([1, CH], f32, tag="stat", name="varrow")
            nc.vector.scalar_tensor_tensor(
                varrow[:, :], S_raw[:, :], 1.0 / (D * D), S_raw[:, :], Alu.mult, Alu.mult)
            nc.vector.scalar_tensor_tensor(
                varrow[:, :], Q_ps[:, :], 1.0 / D, varrow[:, :], Alu.mult, Alu.subtract)
            S_sb = statp.tile([1, CH], f32, tag="stat", name="S_sb")
            nc.vector.tensor_scalar(out=S_sb[:, :], in0=S_raw[:, :],
                                    scalar1=(0.5 * D if shift else 0.0), scalar2=None,
                                    op0=Alu.add)
            return S_sb, varrow

        def ln_st2(x_tile, S_sb, varrow, slot, out_bf16):
            """broadcast mu, center, rstd from the precomputed variance row,
            broadcast, apply (in place on xc for the bf16 path)."""
            bc = psbc.tile([P, CH], f32, tag="ps_bc", name="bc_mu")
            nc.tensor.matmul(bc[:, :], ones_row, S_sb, start=True, stop=True)
            if out_bf16:
                xc = small.tile([P, DT, CH], bf16, tag="small", name="xc")
                for d in range(DT):
                    nc.vector.scalar_tensor_tensor(
                        xc[:, d, :], bc[:, :], -1.0 / D, x_tile[:, d, :],
                        Alu.mult, Alu.add)
            else:
                xc = x_tile
                for d in range(DT):
                    nc.vector.scalar_tensor_tensor(
                        xc[:, d, :], bc[:, :], -1.0 / D, xc[:, d, :], Alu.mult, Alu.add)
            lnv = statp.tile([1, CH], f32, tag="stat", name="lnv")
            nc.scalar.activation(out=lnv[:, :], in_=varrow[:, :], func=Act.Ln,
                                 bias=eps1[:, :], scale=1.0)
            rstd = statp.tile([1, CH], bf16, tag="stat", name="rstd")
            nc.scalar.activation(out=rstd[:, :], in_=lnv[:, :], func=Act.Exp, scale=-0.5)
            bc = psbc.tile([P, CH], f32, tag="ps_bc", name="bc_rstd")
            nc.tensor.matmul(bc[:, :], ones_row_bf[:, :], rstd[:, :], start=True, stop=True)
            a_t = xc if out_bf16 else big.tile([P, DT, CH], f32, tag="big", name="a_t")
            for d in range(DT):
                nc.vector.scalar_tensor_tensor(
                    a_t[:, d, :], xc[:, d, :], lng_sb[:, slot * DT + d: slot * DT + d + 1],
                    bc[:, :], Alu.mult, Alu.mult)
            if has_lnb:
                for d in range(DT):
                    nc.vector.tensor_scalar(
                        out=a_t[:, d, :], in0=a_t[:, d, :],
                        scalar1=lnb_sb[:, slot * DT + d: slot * DT + d + 1], scalar2=None,
                        op0=Alu.add)
            return a_t

        def mlp_chunk(a_t, l, w1_sb, w2_sb, out_tile, out_off):
            hid = hidp.tile([P, HT, CH], bf16, tag="hid", name="hid")
            for mt in range(HT):
                ps = psmm.tile([P, CH], f32, tag="mm", name="ps1")
                for kt in range(DT):
                    nc.tensor.matmul(ps[:, :], w1_sb[:, kt, bass.ts(mt, P)], a_t[:, kt, :],
                                     start=(kt == 0), stop=(kt == DT - 1))
                nc.scalar.activation(out=hid[:, mt, :], in_=ps[:, :], func=Act.Relu,
                                     bias=b1_sb[:, l * HT + mt: l * HT + mt + 1], scale=1.0)
            for mt in range(DT):
                ps = psmm.tile([P, CH], f32, tag="mm", name="ps2")
                for kt in range(HT):
                    nc.tensor.matmul(ps[:, :], w2_sb[:, kt, bass.ts(mt, P)], hid[:, kt, :],
                                     start=(kt == 0), stop=(kt == HT - 1))
                nc.scalar.activation(out=out_tile[:, mt, out_off: out_off + CH], in_=ps[:, :],
                                     func=Act.Identity,
                                     bias=b2_sb[:, l * DT + mt: l * DT + mt + 1], scale=1.0)

        def conv_dw(m_t, l, use_act=False):
            """depthwise conv on a bf16 input tile (bf16 accumulator).
            use_act puts taps 0/1 on ScalarE (activation Copy with
            per-partition scale+bias) to unload DVE in the L0/mid0 region."""
            acc = small.tile([P, DT, CH], bf16, tag="small", name="acc")
            y = small.tile([P, DT, CH], bf16, tag="small", name="y")
            for d in range(DT):
                if use_act:
                    nc.scalar.activation(
                        out=acc[:, d, :], in_=m_t[:, d, 0: CH], func=Act.Identity,
                        scale=dw_sb[:, l * DT + d, 0:1],
                        bias=dwb_sb[:, l * DT + d: l * DT + d + 1])
                    nc.scalar.activation(
                        out=y[:, d, :], in_=m_t[:, d, 1: 1 + CH], func=Act.Identity,
                        scale=dw_sb[:, l * DT + d, 1:2])
                    nc.vector.tensor_add(y[:, d, :], acc[:, d, :], y[:, d, :])
                    for j in range(2, K):
                        nc.vector.scalar_tensor_tensor(
                            y[:, d, :], m_t[:, d, j: j + CH], dw_sb[:, l * DT + d, j: j + 1],
                            y[:, d, :], Alu.mult, Alu.add)
                else:
                    nc.vector.tensor_scalar(
                        out=acc[:, d, :], in0=m_t[:, d, 0: CH],
                        scalar1=dw_sb[:, l * DT + d, 0:1], scalar2=dwb_sb[:, l * DT + d: l * DT + d + 1],
                        op0=Alu.mult, op1=Alu.add)
                    for j in range(1, K - 1):
                        nc.vector.scalar_tensor_tensor(
                            acc[:, d, :], m_t[:, d, j: j + CH], dw_sb[:, l * DT + d, j: j + 1],
                            acc[:, d, :], Alu.mult, Alu.add)
                    nc.vector.scalar_tensor_tensor(
                        y[:, d, :], m_t[:, d, K - 1: K - 1 + CH], dw_sb[:, l * DT + d, K - 1: K],
                        acc[:, d, :], Alu.mult, Alu.add)
            return y

        def conv_pw(y, l, pw_sb):
            cv = small.tile([P, DT, CH], bf16, tag="small", name="cv")
            for mt in range(DT):
                ps = psmm.tile([P, CH], f32, tag="mm", name="ps3")
                for kt in range(DT):
                    nc.tensor.matmul(ps[:, :], pw_sb[:, kt, bass.ts(mt, P)], y[:, kt, :],
                                     start=(kt == 0), stop=(kt == DT - 1))
                nc.scalar.activation(out=cv[:, mt, :], in_=ps[:, :], func=Act.Identity,
                                     bias=pwb_sb[:, l * DT + mt: l * DT + mt + 1], scale=1.0)
            return cv

        def gru_chunk(rhs_bf, res_src, fw_sb, hk_prev, l, res_into_h):
            """kh matmul + gates + scan + residual. The scan state boundary
            column is copied to a tiny keeper tile so the residual add can
            overwrite h in place (mid layers: x_next = h + cv written into
            h fp32) or into the bf16 res_src (L0's bf16 x1 hop).
            Returns (hk, stream_tile)."""
            z = big.tile([P, DT, CH], f32, tag="big", name="z")
            cf = big.tile([P, DT, CH], f32, tag="big", name="cf")
            s = big.tile([P, DT, CH], f32, tag="big", name="s")
            h = big.tile([P, DT, CH], f32, tag="big", name="h")
            for mt in range(MT2):
                ps = psmm.tile([P, CH], f32, tag="mm", name="ps4")
                for kt in range(DT):
                    nc.tensor.matmul(ps[:, :], fw_sb[:, kt, bass.ts(mt, P)], rhs_bf[:, kt, :],
                                     start=(kt == 0), stop=(kt == DT - 1))
                if mt < DT:
                    nc.scalar.activation(out=z[:, mt, :], in_=ps[:, :], func=Act.Sigmoid,
                                         bias=kbz_sb[:, l * MT2 + mt: l * MT2 + mt + 1])
                    nc.scalar.activation(out=cf[:, mt, :], in_=ps[:, :], func=Act.Sigmoid,
                                         bias=kbn_sb[:, l * DT + mt: l * DT + mt + 1],
                                         scale=-1.0)
                else:
                    d = mt - DT
                    nc.scalar.activation(out=s[:, d, :], in_=ps[:, :], func=Act.Sigmoid,
                                         bias=kbz_sb[:, l * MT2 + mt: l * MT2 + mt + 1])
                    nc.vector.scalar_tensor_tensor(
                        s[:, d, :], ps[:, :], kbh_sb[:, l * DT + d: l * DT + d + 1],
                        s[:, d, :], Alu.add, Alu.max)
            for d in range(DT):
                nc.vector.tensor_mul(s[:, d, :], z[:, d, :], s[:, d, :])
            for d in range(DT):
                init = 0.5 if hk_prev is None else hk_prev[:, d, 0:1]
                nc.vector.tensor_tensor_scan(h[:, d, :], cf[:, d, :], s[:, d, :], init,
                                             Alu.mult, Alu.add)
            hk = keep.tile([P, DT, 1], f32, tag="keep", name="hk")
            nc.vector.tensor_copy(out=hk, in_=h[:, :, CH - 1: CH])
            out_t_ = h if res_into_h else res_src
            for d in range(DT):
                nc.vector.tensor_add(out_t_[:, d, :], h[:, d, :], res_src[:, d, :])
            return hk, out_t_

        # ---------- interleaved diagonal-wavefront emission ----------
        # List position p runs stage k at tick p+k. Order: L0c0..3, then
        # (m0c0, L0c4), (m0c1, L0c5), ..., m0c4..7, mid1, mid2, tail.
        # Weight loads are placed so each load's emission follows the
        # previous user's last read of that slot (WAR ordering); fw/pw
        # slots alternate by layer parity (layer l -> slot l%2).
        wds = [{} for _ in range(L - 1)]
        wdt = {}
        st0 = {"h": None}

        def fwtag(l):
            return "fw" + ("A" if l % 2 == 0 else "B")

        def pwtag(l):
            return "pw" + ("A" if l % 2 == 0 else "B")

        def mk_l0(c):
            def s0(_):
                x_in = small.tile([P, DT, CH + 3], bf16, tag="small", name="x_in")
                nc.sync.dma_start(out=x_in, in_=xT.ap().rearrange("(dt p) t -> p dt t", p=P)[:, :, c * CH: c * CH + CH + 3])
                if c == 0:
                    wds[0]["fwA"] = load_w("fwA", fwT, 0, [P, DT, E2])
                    wds[0]["pwA"] = load_w("pwA", pwT, 0, [P, DT, D])
                if c == 1:
                    wds[0]["w1"] = load_w("w1", w1T, 0, [P, DT, H])
                    wds[0]["w2"] = load_w("w2", w2T, 0, [P, HT, D])
                if c == 2:
                    wds[0]["fwB"] = load_w("fwB", fwT, 1, [P, DT, E2])
                    wds[0]["pwB"] = load_w("pwB", pwT, 1, [P, DT, D])
                return conv_dw(x_in, 0, use_act=True)

            def s1(y):
                cv = conv_pw(y, 0, wds[0]["pwA"])
                return (cv,) + ln_stats(cv, shift=False)

            def s2(art):
                cv, S_sb, varrow = art
                n = ln_st2(cv, S_sb, varrow, 0, out_bf16=True)
                return n

            def s3(n):
                st0["h"], res = gru_chunk(n, n, wds[0]["fwA"], st0["h"], 0,
                                          res_into_h=False)
                nc.sync.dma_start(out=dram3(xs_bf, c, CH), in_=res)

            return [s0, s1, s2, s3]

        def mk_mid(c, i):
            wd = wds[i]
            stm = wd.setdefault("st", {"h": None, "m_prev": None})
            src_d, dst_d = xs[i % 2], xs[(i + 1) % 2]

            def s0(_):
                if i >= 1 and c == 2:
                    wd["w1"] = load_w("w1", w1T, i, [P, DT, H])
                    wd["w2"] = load_w("w2", w2T, i, [P, HT, D])
                if i >= 1 and c == 4:
                    wd[fwtag(i + 1)] = load_w(fwtag(i + 1), fwT, i + 1, [P, DT, E2])
                    wd[pwtag(i + 1)] = load_w(pwtag(i + 1), pwT, i + 1, [P, DT, D])
                if i == 0:
                    x_in = small.tile([P, DT, CH], bf16, tag="small", name="x_in")
                    nc.sync.dma_start(out=x_in, in_=dram3(xs_bf, c, CH))
                    return (x_in,) + ln_stats(x_in, shift=False)
                x_in = big.tile([P, DT, CH], f32, tag="big", name="x_in")
                nc.sync.dma_start(out=x_in, in_=dram3(src_d, c, CH))
                xb = small.tile([P, DT, CH], bf16, tag="small", name="xb")
                for d in range(DT):
                    nc.scalar.activation(out=xb[:, d, :], in_=x_in[:, d, :],
                                         func=Act.Copy, bias=-0.5)
                return (x_in,) + ln_stats(xb, shift=True)

            def s1(art):
                x_in, S_sb, varrow = art
                return ln_st2(x_in, S_sb, varrow, 1 + i, out_bf16=True)

            def s2(a):
                m = small.tile([P, DT, CH + 3], bf16, tag="small", name="m")
                mlp_chunk(a, i, wd["w1"], wd["w2"], m, 3)
                if c == 0:
                    nc.vector.memset(m[:, :, 0:3], 0.0)
                else:
                    nc.vector.tensor_copy(out=m[:, :, 0:3], in_=stm["m_prev"][:, :, CH: CH + 3])
                stm["m_prev"] = m
                return m

            def s3(m):
                return conv_dw(m, i + 1, use_act=(i == 0))

            def s4(y):
                cv = conv_pw(y, i + 1, wd[pwtag(i + 1)])
                stm["h"], res = gru_chunk(y, cv, wd[fwtag(i + 1)], stm["h"], i + 1,
                                          res_into_h=True)
                nc.sync.dma_start(out=dram3(dst_d, c, CH), in_=res)

            return [s0, s1, s2, s3, s4]

        src_t = xs[(L - 1) % 2]

        def mk_tail(c):
            def s0(_):
                if c == 2:
                    wdt["w1"] = load_w("w1", w1T, L - 1, [P, DT, H])
                    wdt["w2"] = load_w("w2", w2T, L - 1, [P, HT, D])
                x_in = big.tile([P, DT, CH], f32, tag="big", name="x_in")
                nc.sync.dma_start(out=x_in, in_=dram3(src_t, c, CH))
                xb = small.tile([P, DT, CH], bf16, tag="small", name="xb")
                for d in range(DT):
                    nc.scalar.activation(out=xb[:, d, :], in_=x_in[:, d, :],
                                         func=Act.Copy, bias=-0.5)
                return (x_in,) + ln_stats(xb, shift=True)

            def s1(art):
                x_in, S_sb, varrow = art
                return ln_st2(x_in, S_sb, varrow, L, out_bf16=True)

            def s2(a):
                o = big.tile([P, DT, CH], f32, tag="big", name="o")
                mlp_chunk(a, L - 1, wdt["w1"], wdt["w2"], o, 0)
                nc.sync.dma_start(out=dram3(out_t, c, CH), in_=o)

            return [s0, s1, s2]

        l0c = [mk_l0(c) for c in range(NCH)]
        m0c = [mk_mid(c, 0) for c in range(NCH)]
        chunks = l0c[0:4]
        for j in range(4):
            chunks += [m0c[j], l0c[4 + j]]
        chunks += m0c[4:]
        for i in range(1, L - 1):
            chunks += [mk_mid(c, i) for c in range(NCH)]
        chunks += [mk_tail(c) for c in range(NCH)]

        NST = 5
        arts = [None] * len(chunks)
        for g in range(len(chunks) + NST - 1):
            for k in (0, 1, 2, 4, 3):
                idx = g - k
                if 0 <= idx < len(chunks) and k < len(chunks[idx]):
                    arts[idx] = chunks[idx][k](arts[idx])

    return nc


_CACHE = {}


def get_compiled_nc(T=4096, CH=512, has_lnb=False, **kw):
    key = (T, CH, has_lnb, tuple(sorted(kw.items())))
    if key not in _CACHE:
        nc = build_nc(T, CH, has_lnb, **kw)
        nc.compile()
        _CACHE[key] = nc
    return _CACHE[key]


def _part3(a):
    """[Kdim, E] -> [P, Kdim//P, E] partition-contiguous host layout."""
    Kd, E = a.shape
    return np.ascontiguousarray(a.reshape(Kd // P, P, E).transpose(1, 0, 2))


def _rows(a):
    """[L?, D?] -> [P, L*DT] host layout (row l*DT+dt holds a[l, dt*128+p])."""
    Ld, Dd = a.shape
    return np.ascontiguousarray(a.reshape(Ld, Dd // P, P).transpose(2, 0, 1).reshape(P, -1))


def make_host_inputs(inputs, T=4096):
    f = np.float32
    DT, HT, E2 = D // P, H // P, 2 * D
    f_w = np.asarray(inputs["f_w"], f)
    pw_w = np.asarray(inputs["conv_pw_w"], f)
    pw_b = np.asarray(inputs["conv_pw_b"], f)
    # fold conv pointwise into the GRU input projection for layers 1..L-1
    fw_eff = [f_w[0]] + [f_w[l] @ pw_w[l] for l in range(1, L)]
    kb = np.stack([np.zeros(E2, f)] + [f_w[l] @ pw_b[l] for l in range(1, L)])
    w = {
        "fwT": np.stack([_part3(m.T) for m in fw_eff]).astype(BF),
        "pwT": np.stack([_part3(pw_w[l].T) for l in range(L)]).astype(BF),
        "w1T": np.stack([_part3(np.asarray(inputs["mlp_w1"], f)[l].T) for l in range(L)]).astype(BF),
        "w2T": np.stack([_part3(np.asarray(inputs["mlp_w2"], f)[l].T) for l in range(L)]).astype(BF),
        "dwK": np.ascontiguousarray(
            np.asarray(inputs["conv_dw_w"], f).transpose(0, 2, 1)  # [L, D, K]
            .reshape(L, DT, P, K).transpose(2, 0, 1, 3).reshape(P, L * DT, K)),
        "dwb": _rows(np.asarray(inputs["conv_dw_b"], f)),
        "pwb": _rows(pw_b),
        "b1v": _rows(np.asarray(inputs["mlp_b1"], f)),
        "b2v": _rows(np.asarray(inputs["mlp_b2"], f)),
        "lng": _rows(np.concatenate([np.asarray(inputs["ln1_g"], f)[None], np.asarray(inputs["ln2_g"], f)], 0)),
        "lnb": _rows(np.concatenate([np.asarray(inputs["ln1_b"], f)[None], np.asarray(inputs["ln2_b"], f)], 0)),
        "kbz": _rows(kb),
        "kbn": _rows(-kb[:, :D]),
        "kbh": _rows(kb[:, D:] + 0.5),
    }
    x = np.asarray(inputs["x"], f)
    nb = x.shape[0]
    in_maps = []
    for b in range(nb):
        xTp = np.zeros((D, T + 3), f)
        xTp[:, 3:] = x[b, :T].T
        in_maps.append({"xT": xTp.astype(BF), **w})
    has_lnb = bool(np.any(w["lnb"] != 0.0))
    return in_maps, has_lnb


def kernel(**inputs):
    from concourse.bass_utils import run_bass_kernel_spmd

    T = int(np.asarray(inputs["x"]).shape[1])
    in_maps, has_lnb = make_host_inputs(inputs, T)
    nc = get_compiled_nc(T=T, has_lnb=has_lnb)
    res = run_bass_kernel_spmd(nc, in_maps, core_ids=list(range(len(in_maps))))
    out = np.stack([r["out"].T for r in res.results])
    return np.ascontiguousarray(out.astype(np.float32))



# revision 12
# speedup vs baseline: 1.0333x; 1.0333x over previous
"""Trainium2 Bass kernel for nn_BlockV2 (conv -> LN -> minGRU -> MLP x4).

Strategy: data-parallel over batch (B=8 -> 8 cores). Per core, activations
are kept in [D_partitions, T_free] layout and streamed through each layer in
chunks of 512 tokens; inter-layer activations ping-pong through DRAM in
fp32 (the late-layer stream is a ~5e-3 variation on an O(1) baseline;
storing it bf16 measures 28% output error - LN divides by the tiny
per-token sigma and amplifies absolute quantization noise ~200x).
Values that carry *relative* information are bf16: the centered LN values,
the conv taps/accumulator, the MLP output m, and the raw input x.
The minGRU recurrence h_t = c_t*h_{t-1} + v_t runs on the VectorE
tensor_tensor_scan instruction (fp32 state), chained across chunks.

Perf structure (v4):
- conv pointwise folded into the GRU input projection on the host
  (FW2 = f_w @ pw, bias into the gate activations): the kh matmul streams
  the depthwise-conv output y directly.
- LN mean sums run as bf16 matmuls over a bf16 copy of x fetched by a
  GpSimd casting DMA (mean over 512 washes out the quantization noise);
  fp32 4-cyc/row stat matmuls remain only for L0's SBUF-resident cv.
- Ln/Exp are routed to the combined natural_log_exp_and_others table set
  (the default per-function choice loads natural_log and exp_and_others
  separately - 3 table swaps per chunk instead of 2).
- Mid-layer-0 chunks interleave into the L0 chunk stream (stagger 4): L0
  alone is PE-light/vector-heavy and left TensorE at 22-67% occupancy for
  350us. fw/pw weight slots alternate by layer parity so the prefetch
  cannot WAR-deadlock against the previous layer's last reads.
- All small parameters are pre-swizzled on the host so DMAs are
  partition-contiguous.
"""
import sys

sys.path.insert(0, "/opt/trn_rl_repo")

from contextlib import ExitStack

import numpy as np
import ml_dtypes

import concourse.bass as bass
import concourse.bass_isa as bass_isa
import concourse.tile as tile
from concourse import bacc, mybir

# Route Ln/Exp activations to the combined natural_log_exp_and_others table
# set (set order, and thus act_func_set_id, is preserved; only membership of
# the single-anchor sets is trimmed).
import functools
import concourse.hw_specs as _hw_specs

_orig_gat = _hw_specs.get_activation_tables


@functools.cache
def _patched_gat(arch):
    t = {k: set(v) for k, v in _orig_gat(arch).items()}
    comb = "natural_log_exp_and_others"
    if comb in t:
        for name, fns in t.items():
            if name != comb:
                fns.discard(mybir.ActivationFunctionType.Ln)
                fns.discard(mybir.ActivationFunctionType.Exp)
    return t


_hw_specs.get_activation_tables = _patched_gat
bacc.get_activation_tables = _patched_gat

f32 = mybir.dt.float32
bf16 = mybir.dt.bfloat16
Alu = mybir.AluOpType
Act = mybir.ActivationFunctionType
BF = ml_dtypes.bfloat16

B, D, L, K, H = 8, 512, 4, 4, 2048
N_CORES = 8
LN_EPS = 1e-5
P = 128


def build_nc(T=4096, CH=512, has_lnb=False):
    NCH = T // CH
    DT = D // P      # 4 d-tiles
    HT = H // P      # 16 h-tiles
    E2 = 2 * D
    MT2 = E2 // P    # 8 m-tiles of the kh matmul

    nc = bacc.Bacc("TRN2", target_bir_lowering=False, debug=False)

    xT = nc.dram_tensor("xT", [D, T + 3], bf16, kind="ExternalInput")
    fwT = nc.dram_tensor("fwT", [L, P, DT, E2], bf16, kind="ExternalInput")
    pwT = nc.dram_tensor("pwT", [L, P, DT, D], bf16, kind="ExternalInput")
    w1T = nc.dram_tensor("w1T", [L, P, DT, H], bf16, kind="ExternalInput")
    w2T = nc.dram_tensor("w2T", [L, P, HT, D], bf16, kind="ExternalInput")
    dwK = nc.dram_tensor("dwK", [P, L * DT, K], f32, kind="ExternalInput")
    dwb = nc.dram_tensor("dwb", [P, L * DT], f32, kind="ExternalInput")
    pwb = nc.dram_tensor("pwb", [P, L * DT], f32, kind="ExternalInput")
    b1v = nc.dram_tensor("b1v", [P, L * HT], f32, kind="ExternalInput")
    b2v = nc.dram_tensor("b2v", [P, L * DT], f32, kind="ExternalInput")
    lng = nc.dram_tensor("lng", [P, (L + 1) * DT], f32, kind="ExternalInput")
    lnb = nc.dram_tensor("lnb", [P, (L + 1) * DT], f32, kind="ExternalInput")
    kbz = nc.dram_tensor("kbz", [P, L * MT2], f32, kind="ExternalInput")
    kbn = nc.dram_tensor("kbn", [P, L * DT], f32, kind="ExternalInput")
    kbh = nc.dram_tensor("kbh", [P, L * DT], f32, kind="ExternalInput")
    out_t = nc.dram_tensor("out", [D, T], f32, kind="ExternalOutput")
    xs = [nc.dram_tensor(f"xs{i}", [D, T], f32) for i in range(2)]
    xs_bf = nc.dram_tensor("xs_bf", [D, T], bf16)

    def dram3(tensor, c, width):
        return tensor.ap().rearrange("(dt p) t -> p dt t", p=P)[:, :, c * CH: c * CH + width]

    with tile.TileContext(nc) as tc, ExitStack() as ctx:
        sing = ctx.enter_context(tc.tile_pool(name="sing", bufs=1))
        wpool = ctx.enter_context(tc.tile_pool(name="w", bufs=1))
        big = ctx.enter_context(tc.tile_pool(name="big", bufs=7))
        small = ctx.enter_context(tc.tile_pool(name="small", bufs=10))
        hidp = ctx.enter_context(tc.tile_pool(name="hid", bufs=2))
        statq = ctx.enter_context(tc.tile_pool(name="statq", bufs=3))
        statv = ctx.enter_context(tc.tile_pool(name="statv", bufs=2))
        statr = ctx.enter_context(tc.tile_pool(name="statr", bufs=3))
        keep = ctx.enter_context(tc.tile_pool(name="keep", bufs=4))
        psmm = ctx.enter_context(tc.tile_pool(name="psmm", bufs=8, space="PSUM"))

        eps_col = sing.tile([P, 1], f32)
        nc.vector.memset(eps_col, LN_EPS)
        dw_sb = sing.tile([P, L * DT, K], f32)
        nc.sync.dma_start(out=dw_sb, in_=dwK.ap())
        dwb_sb = sing.tile([P, L * DT], f32)
        nc.sync.dma_start(out=dwb_sb, in_=dwb.ap())
        pwb_sb = sing.tile([P, L * DT], f32)
        nc.sync.dma_start(out=pwb_sb, in_=pwb.ap())
        b1_sb = sing.tile([P, L * HT], f32)
        nc.sync.dma_start(out=b1_sb, in_=b1v.ap())
        b2_sb = sing.tile([P, L * DT], f32)
        nc.sync.dma_start(out=b2_sb, in_=b2v.ap())
        lng_sb = sing.tile([P, (L + 1) * DT], f32)
        nc.sync.dma_start(out=lng_sb, in_=lng.ap())
        lnb_sb = sing.tile([P, (L + 1) * DT], f32)
        nc.sync.dma_start(out=lnb_sb, in_=lnb.ap())
        kbz_sb = sing.tile([P, L * MT2], f32)
        nc.sync.dma_start(out=kbz_sb, in_=kbz.ap())
        kbn_sb = sing.tile([P, L * DT], f32)
        nc.sync.dma_start(out=kbn_sb, in_=kbn.ap())
        kbh_sb = sing.tile([P, L * DT], f32)
        nc.sync.dma_start(out=kbh_sb, in_=kbh.ap())

        def load_w(tag, dram, l, shape):
            t = wpool.tile(shape, bf16, tag=tag, name=f"{tag}_{l}")
            nc.sync.dma_start(out=t, in_=dram.ap()[l])
            return t

        def ln_stats(xq, shift):
            """s0 stats off the PE: dt-axis partial sums of x and x^2 on DVE,
            then the 128-partition sums via a single in-place GpSimd
            partition_all_reduce whose output is already broadcast across
            partitions. var = Q/D - (S/D)^2 (benign after the -0.5 shift of
            the fragile mid streams) lands in the Q slot in place. Returns
            the [P, 2, CH] tile: slot 0 = S_bc (incl. shift-back), slot 1 =
            varv."""
            xsq = small.tile([P, DT, CH], bf16, tag="small", name="xsq")
            for d in range(DT):
                nc.vector.tensor_mul(xsq[:, d, :], xq[:, d, :], xq[:, d, :])
            SQ = statq.tile([P, 2, CH], f32, tag="statq", name="SQ")
            nc.vector.tensor_add(SQ[:, 0, :], xq[:, 0, :], xq[:, 1, :])
            nc.vector.tensor_add(SQ[:, 1, :], xsq[:, 0, :], xsq[:, 1, :])
            nc.vector.tensor_add(SQ[:, 0, :], SQ[:, 0, :], xq[:, 2, :])
            nc.vector.tensor_add(SQ[:, 1, :], SQ[:, 1, :], xsq[:, 2, :])
            nc.vector.tensor_add(SQ[:, 0, :], SQ[:, 0, :], xq[:, 3, :])
            nc.vector.tensor_add(SQ[:, 1, :], SQ[:, 1, :], xsq[:, 3, :])
            nc.gpsimd.partition_all_reduce(SQ[:, :, :], SQ[:, :, :], channels=P,
                                           reduce_op=bass_isa.ReduceOp.add)
            S_bc, Q_bc = SQ[:, 0, :], SQ[:, 1, :]
            tsq = statv.tile([P, CH], f32, tag="statv", name="tsq")
            nc.vector.scalar_tensor_tensor(
                tsq, S_bc, 1.0 / (D * D), S_bc, Alu.mult, Alu.mult)
            nc.vector.scalar_tensor_tensor(
                Q_bc, Q_bc, 1.0 / D, tsq, Alu.mult, Alu.subtract)
            if shift:
                nc.vector.tensor_scalar(out=S_bc, in0=S_bc, scalar1=0.5 * D,
                                        scalar2=None, op0=Alu.add)
            return SQ

        def ln_st2(x_tile, SQ, slot, out_bf16):
            """center with the broadcast mean, rstd from the variance slot,
            apply (in place on xc)."""
            S_bc, varv = SQ[:, 0, :], SQ[:, 1, :]
            xc = small.tile([P, DT, CH], bf16, tag="small", name="xc")
            for d in range(DT):
                nc.vector.scalar_tensor_tensor(
                    xc[:, d, :], S_bc, -1.0 / D, x_tile[:, d, :],
                    Alu.mult, Alu.add)
            nc.scalar.activation(out=varv, in_=varv, func=Act.Ln,
                                 bias=eps_col[:, :], scale=1.0)
            rstd = statr.tile([P, CH], bf16, tag="statr", name="rstd")
            nc.scalar.activation(out=rstd, in_=varv, func=Act.Exp, scale=-0.5)
            a_t = xc
            for d in range(DT):
                nc.vector.scalar_tensor_tensor(
                    a_t[:, d, :], xc[:, d, :], lng_sb[:, slot * DT + d: slot * DT + d + 1],
                    rstd, Alu.mult, Alu.mult)
            if has_lnb:
                for d in range(DT):
                    nc.vector.tensor_scalar(
                        out=a_t[:, d, :], in0=a_t[:, d, :],
                        scalar1=lnb_sb[:, slot * DT + d: slot * DT + d + 1], scalar2=None,
                        op0=Alu.add)
            return a_t

        def mlp_chunk(a_t, l, w1_sb, w2_sb, out_tile, out_off):
            hid = hidp.tile([P, HT, CH], bf16, tag="hid", name="hid")
            for mt in range(HT):
                ps = psmm.tile([P, CH], f32, tag="mm", name="ps1")
                for kt in range(DT):
                    nc.tensor.matmul(ps[:, :], w1_sb[:, kt, bass.ts(mt, P)], a_t[:, kt, :],
                                     start=(kt == 0), stop=(kt == DT - 1))
                nc.scalar.activation(out=hid[:, mt, :], in_=ps[:, :], func=Act.Relu,
                                     bias=b1_sb[:, l * HT + mt: l * HT + mt + 1], scale=1.0)
            for mt in range(DT):
                ps = psmm.tile([P, CH], f32, tag="mm", name="ps2")
                for kt in range(HT):
                    nc.tensor.matmul(ps[:, :], w2_sb[:, kt, bass.ts(mt, P)], hid[:, kt, :],
                                     start=(kt == 0), stop=(kt == HT - 1))
                nc.scalar.activation(out=out_tile[:, mt, out_off: out_off + CH], in_=ps[:, :],
                                     func=Act.Identity,
                                     bias=b2_sb[:, l * DT + mt: l * DT + mt + 1], scale=1.0)

        def conv_dw(m_t, l, use_act=False):
            """depthwise conv on a bf16 input tile (bf16 accumulator).
            use_act puts taps 0/1 on ScalarE (activation Copy with
            per-partition scale+bias) to unload DVE in the L0/mid0 region."""
            acc = small.tile([P, DT, CH], bf16, tag="small", name="acc")
            y = small.tile([P, DT, CH], bf16, tag="small", name="y")
            for d in range(DT):
                if use_act:
                    nc.scalar.activation(
                        out=acc[:, d, :], in_=m_t[:, d, 0: CH], func=Act.Identity,
                        scale=dw_sb[:, l * DT + d, 0:1],
                        bias=dwb_sb[:, l * DT + d: l * DT + d + 1])
                    nc.scalar.activation(
                        out=y[:, d, :], in_=m_t[:, d, 1: 1 + CH], func=Act.Identity,
                        scale=dw_sb[:, l * DT + d, 1:2])
                    nc.vector.tensor_add(y[:, d, :], acc[:, d, :], y[:, d, :])
                    for j in range(2, K):
                        nc.vector.scalar_tensor_tensor(
                            y[:, d, :], m_t[:, d, j: j + CH], dw_sb[:, l * DT + d, j: j + 1],
                            y[:, d, :], Alu.mult, Alu.add)
                else:
                    nc.vector.tensor_scalar(
                        out=acc[:, d, :], in0=m_t[:, d, 0: CH],
                        scalar1=dw_sb[:, l * DT + d, 0:1], scalar2=dwb_sb[:, l * DT + d: l * DT + d + 1],
                        op0=Alu.mult, op1=Alu.add)
                    for j in range(1, K - 1):
                        nc.vector.scalar_tensor_tensor(
                            acc[:, d, :], m_t[:, d, j: j + CH], dw_sb[:, l * DT + d, j: j + 1],
                            acc[:, d, :], Alu.mult, Alu.add)
                    nc.vector.scalar_tensor_tensor(
                        y[:, d, :], m_t[:, d, K - 1: K - 1 + CH], dw_sb[:, l * DT + d, K - 1: K],
                        acc[:, d, :], Alu.mult, Alu.add)
            return y

        def conv_pw(y, l, pw_sb):
            cv = small.tile([P, DT, CH], bf16, tag="small", name="cv")
            for mt in range(DT):
                ps = psmm.tile([P, CH], f32, tag="mm", name="ps3")
                for kt in range(DT):
                    nc.tensor.matmul(ps[:, :], pw_sb[:, kt, bass.ts(mt, P)], y[:, kt, :],
                                     start=(kt == 0), stop=(kt == DT - 1))
                nc.scalar.activation(out=cv[:, mt, :], in_=ps[:, :], func=Act.Identity,
                                     bias=pwb_sb[:, l * DT + mt: l * DT + mt + 1], scale=1.0)
            return cv

        def gru_chunk(rhs_bf, res_src, fw_sb, hk_prev, l, res_into_h):
            """kh matmul + gates + scan + residual. The scan state boundary
            column is copied to a tiny keeper tile so the residual add can
            overwrite h in place (mid layers: x_next = h + cv written into
            h fp32) or into the bf16 res_src (L0's bf16 x1 hop).
            Returns (hk, stream_tile)."""
            z = big.tile([P, DT, CH], f32, tag="big", name="z")
            cf = big.tile([P, DT, CH], f32, tag="big", name="cf")
            s = big.tile([P, DT, CH], f32, tag="big", name="s")
            h = big.tile([P, DT, CH], f32, tag="big", name="h")
            for mt in range(MT2):
                ps = psmm.tile([P, CH], f32, tag="mm", name="ps4")
                for kt in range(DT):
                    nc.tensor.matmul(ps[:, :], fw_sb[:, kt, bass.ts(mt, P)], rhs_bf[:, kt, :],
                                     start=(kt == 0), stop=(kt == DT - 1))
                if mt < DT:
                    nc.scalar.activation(out=z[:, mt, :], in_=ps[:, :], func=Act.Sigmoid,
                                         bias=kbz_sb[:, l * MT2 + mt: l * MT2 + mt + 1])
                    nc.scalar.activation(out=cf[:, mt, :], in_=ps[:, :], func=Act.Sigmoid,
                                         bias=kbn_sb[:, l * DT + mt: l * DT + mt + 1],
                                         scale=-1.0)
                else:
                    d = mt - DT
                    nc.scalar.activation(out=s[:, d, :], in_=ps[:, :], func=Act.Sigmoid,
                                         bias=kbz_sb[:, l * MT2 + mt: l * MT2 + mt + 1])
                    nc.vector.scalar_tensor_tensor(
                        s[:, d, :], ps[:, :], kbh_sb[:, l * DT + d: l * DT + d + 1],
                        s[:, d, :], Alu.add, Alu.max)
            for d in range(DT):
                nc.vector.tensor_mul(s[:, d, :], z[:, d, :], s[:, d, :])
            for d in range(DT):
                init = 0.5 if hk_prev is None else hk_prev[:, d, 0:1]
                nc.vector.tensor_tensor_scan(h[:, d, :], cf[:, d, :], s[:, d, :], init,
                                             Alu.mult, Alu.add)
            hk = keep.tile([P, DT, 1], f32, tag="keep", name="hk")
            nc.vector.tensor_copy(out=hk, in_=h[:, :, CH - 1: CH])
            out_t_ = h if res_into_h else res_src
            for d in range(DT):
                nc.vector.tensor_add(out_t_[:, d, :], h[:, d, :], res_src[:, d, :])
            return hk, out_t_

        # ---------- interleaved diagonal-wavefront emission ----------
        # List position p runs stage k at tick p+k. Order: L0c0..3, then
        # (m0c0, L0c4), (m0c1, L0c5), ..., m0c4..7, mid1, mid2, tail.
        # Weight loads are placed so each load's emission follows the
        # previous user's last read of that slot (WAR ordering); fw/pw
        # slots alternate by layer parity (layer l -> slot l%2).
        wds = [{} for _ in range(L - 1)]
        wdt = {}
        st0 = {"h": None}

        def fwtag(l):
            return "fw" + ("A" if l % 2 == 0 else "B")

        def pwtag(l):
            return "pw" + ("A" if l % 2 == 0 else "B")

        def mk_l0(c):
            def s0(_):
                x_in = small.tile([P, DT, CH + 3], bf16, tag="small", name="x_in")
                nc.sync.dma_start(out=x_in, in_=xT.ap().rearrange("(dt p) t -> p dt t", p=P)[:, :, c * CH: c * CH + CH + 3])
                if c == 0:
                    wds[0]["fwA"] = load_w("fwA", fwT, 0, [P, DT, E2])
                    wds[0]["pwA"] = load_w("pwA", pwT, 0, [P, DT, D])
                if c == 1:
                    wds[0]["w1"] = load_w("w1", w1T, 0, [P, DT, H])
                    wds[0]["w2"] = load_w("w2", w2T, 0, [P, HT, D])
                if c == 2:
                    wds[0]["fwB"] = load_w("fwB", fwT, 1, [P, DT, E2])
                    wds[0]["pwB"] = load_w("pwB", pwT, 1, [P, DT, D])
                return conv_dw(x_in, 0, use_act=True)

            def s1(y):
                cv = conv_pw(y, 0, wds[0]["pwA"])
                return (cv, ln_stats(cv, shift=False))

            def s2(art):
                cv, SQ = art
                n = ln_st2(cv, SQ, 0, out_bf16=True)
                return n

            def s3(n):
                st0["h"], res = gru_chunk(n, n, wds[0]["fwA"], st0["h"], 0,
                                          res_into_h=False)
                nc.sync.dma_start(out=dram3(xs_bf, c, CH), in_=res)

            return [s0, s1, s2, s3]

        def mk_mid(c, i):
            wd = wds[i]
            stm = wd.setdefault("st", {"h": None, "m_prev": None})
            src_d, dst_d = xs[i % 2], xs[(i + 1) % 2]

            def s0(_):
                if i >= 1 and c == 2:
                    wd["w1"] = load_w("w1", w1T, i, [P, DT, H])
                    wd["w2"] = load_w("w2", w2T, i, [P, HT, D])
                if i >= 1 and c == 4:
                    wd[fwtag(i + 1)] = load_w(fwtag(i + 1), fwT, i + 1, [P, DT, E2])
                    wd[pwtag(i + 1)] = load_w(pwtag(i + 1), pwT, i + 1, [P, DT, D])
                if i == 0:
                    x_in = small.tile([P, DT, CH], bf16, tag="small", name="x_in")
                    nc.sync.dma_start(out=x_in, in_=dram3(xs_bf, c, CH))
                    return (x_in, ln_stats(x_in, shift=False))
                x_in = big.tile([P, DT, CH], f32, tag="big", name="x_in")
                nc.sync.dma_start(out=x_in, in_=dram3(src_d, c, CH))
                xb = small.tile([P, DT, CH], bf16, tag="small", name="xb")
                for d in range(DT):
                    nc.scalar.activation(out=xb[:, d, :], in_=x_in[:, d, :],
                                         func=Act.Copy, bias=-0.5)
                return (x_in, ln_stats(xb, shift=True))

            def s1(art):
                x_in, SQ = art
                return ln_st2(x_in, SQ, 1 + i, out_bf16=True)

            def s2(a):
                m = small.tile([P, DT, CH + 3], bf16, tag="small", name="m")
                mlp_chunk(a, i, wd["w1"], wd["w2"], m, 3)
                if c == 0:
                    nc.vector.memset(m[:, :, 0:3], 0.0)
                else:
                    nc.vector.tensor_copy(out=m[:, :, 0:3], in_=stm["m_prev"][:, :, CH: CH + 3])
                stm["m_prev"] = m
                return m

            def s3(m):
                return conv_dw(m, i + 1, use_act=(i == 0))

            def s4(y):
                cv = conv_pw(y, i + 1, wd[pwtag(i + 1)])
                stm["h"], res = gru_chunk(y, cv, wd[fwtag(i + 1)], stm["h"], i + 1,
                                          res_into_h=True)
                nc.sync.dma_start(out=dram3(dst_d, c, CH), in_=res)

            return [s0, s1, s2, s3, s4]

        src_t = xs[(L - 1) % 2]

        def mk_tail(c):
            def s0(_):
                if c == 2:
                    wdt["w1"] = load_w("w1", w1T, L - 1, [P, DT, H])
                    wdt["w2"] = load_w("w2", w2T, L - 1, [P, HT, D])
                x_in = big.tile([P, DT, CH], f32, tag="big", name="x_in")
                nc.sync.dma_start(out=x_in, in_=dram3(src_t, c, CH))
                xb = small.tile([P, DT, CH], bf16, tag="small", name="xb")
                for d in range(DT):
                    nc.scalar.activation(out=xb[:, d, :], in_=x_in[:, d, :],
                                         func=Act.Copy, bias=-0.5)
                return (x_in, ln_stats(xb, shift=True))

            def s1(art):
                x_in, SQ = art
                return ln_st2(x_in, SQ, L, out_bf16=True)

            def s2(a):
                o = big.tile([P, DT, CH], f32, tag="big", name="o")
                mlp_chunk(a, L - 1, wdt["w1"], wdt["w2"], o, 0)
                nc.sync.dma_start(out=dram3(out_t, c, CH), in_=o)

            return [s0, s1, s2]

        l0c = [mk_l0(c) for c in range(NCH)]
        m0c = [mk_mid(c, 0) for c in range(NCH)]
        chunks = l0c[0:4]
        for j in range(4):
            chunks += [m0c[j], l0c[4 + j]]
        chunks += m0c[4:]
        for i in range(1, L - 1):
            chunks += [mk_mid(c, i) for c in range(NCH)]
        chunks += [mk_tail(c) for c in range(NCH)]

        NST = 5
        arts = [None] * len(chunks)
        for g in range(len(chunks) + NST - 1):
            for k in (0, 1, 2, 4, 3):
                idx = g - k
                if 0 <= idx < len(chunks) and k < len(chunks[idx]):
                    arts[idx] = chunks[idx][k](arts[idx])

    return nc


_CACHE = {}


def get_compiled_nc(T=4096, CH=512, has_lnb=False, **kw):
    key = (T, CH, has_lnb, tuple(sorted(kw.items())))
    if key not in _CACHE:
        nc = build_nc(T, CH, has_lnb, **kw)
        nc.compile()
        _CACHE[key] = nc
    return _CACHE[key]


def _part3(a):
    """[Kdim, E] -> [P, Kdim//P, E] partition-contiguous host layout."""
    Kd, E = a.shape
    return np.ascontiguousarray(a.reshape(Kd // P, P, E).transpose(1, 0, 2))


def _rows(a):
    """[L?, D?] -> [P, L*DT] host layout (row l*DT+dt holds a[l, dt*128+p])."""
    Ld, Dd = a.shape
    return np.ascontiguousarray(a.reshape(Ld, Dd // P, P).transpose(2, 0, 1).reshape(P, -1))


def make_host_inputs(inputs, T=4096):
    f = np.float32
    DT, HT, E2 = D // P, H // P, 2 * D
    f_w = np.asarray(inputs["f_w"], f)
    pw_w = np.asarray(inputs["conv_pw_w"], f)
    pw_b = np.asarray(inputs["conv_pw_b"], f)
    # fold conv pointwise into the GRU input projection for layers 1..L-1
    fw_eff = [f_w[0]] + [f_w[l] @ pw_w[l] for l in range(1, L)]
    kb = np.stack([np.zeros(E2, f)] + [f_w[l] @ pw_b[l] for l in range(1, L)])
    w = {
        "fwT": np.stack([_part3(m.T) for m in fw_eff]).astype(BF),
        "pwT": np.stack([_part3(pw_w[l].T) for l in range(L)]).astype(BF),
        "w1T": np.stack([_part3(np.asarray(inputs["mlp_w1"], f)[l].T) for l in range(L)]).astype(BF),
        "w2T": np.stack([_part3(np.asarray(inputs["mlp_w2"], f)[l].T) for l in range(L)]).astype(BF),
        "dwK": np.ascontiguousarray(
            np.asarray(inputs["conv_dw_w"], f).transpose(0, 2, 1)  # [L, D, K]
            .reshape(L, DT, P, K).transpose(2, 0, 1, 3).reshape(P, L * DT, K)),
        "dwb": _rows(np.asarray(inputs["conv_dw_b"], f)),
        "pwb": _rows(pw_b),
        "b1v": _rows(np.asarray(inputs["mlp_b1"], f)),
        "b2v": _rows(np.asarray(inputs["mlp_b2"], f)),
        "lng": _rows(np.concatenate([np.asarray(inputs["ln1_g"], f)[None], np.asarray(inputs["ln2_g"], f)], 0)),
        "lnb": _rows(np.concatenate([np.asarray(inputs["ln1_b"], f)[None], np.asarray(inputs["ln2_b"], f)], 0)),
        "kbz": _rows(kb),
        "kbn": _rows(-kb[:, :D]),
        "kbh": _rows(kb[:, D:] + 0.5),
    }
    x = np.asarray(inputs["x"], f)
    nb = x.shape[0]
    in_maps = []
    for b in range(nb):
        xTp = np.zeros((D, T + 3), f)
        xTp[:, 3:] = x[b, :T].T
        in_maps.append({"xT": xTp.astype(BF), **w})
    has_lnb = bool(np.any(w["lnb"] != 0.0))
    return in_maps, has_lnb


def kernel(**inputs):
    from concourse.bass_utils import run_bass_kernel_spmd

    T = int(np.asarray(inputs["x"]).shape[1])
    in_maps, has_lnb = make_host_inputs(inputs, T)
    nc = get_compiled_nc(T=T, has_lnb=has_lnb)
    res = run_bass_kernel_spmd(nc, in_maps, core_ids=list(range(len(in_maps))))
    out = np.stack([r["out"].T for r in res.results])
    return np.ascontiguousarray(out.astype(np.float32))

